# revision 23
# baseline (speedup 1.0000x reference)
"""Trainium2 Bass kernel for nn_EpisodicMemory (scatter_memory).

Computation (reference):
  x  = query.reshape(-1, 14)                      # [N, 14], N = 4*65536
  h  = gelu(x @ W1.T + b1)                        # [N, 32]
  q8 = h @ W2.T + b2                              # [N, 8]
  q8n= q8 / max(||q8||, 1e-12)
  sim= q8n @ keys_n.T                             # [N, 240]
  attn = softmax(2.0 * sim)                       # [N, 240]
  content = attn @ values                         # [N, 256]
  return content, attn

Sharding: data-parallel over rows, N/8 = 32768 rows per NeuronCore.

Per-core dataflow (two ACT-table passes to avoid gelu<->exp table thrash):
  pass 1 (gelu set):  DMA x -> PE transpose -> mm1 -> gelu -> mm2
                      -> q8 parked in SBUF [8, 32768]; per-chunk |q8|^2
                      collected into one PSUM [128, 256] tile (col = chunk).
  prelude:            rnorm2 = beta/sqrt(normsq) = exp(-0.5*ln(normsq)+ln beta)
                      batched for all rows: one Ln + one Exp total.
  pass 2 (ln/exp set): sim rows [128,240] x4 from raw q8; exp applies
                      the rnorm2 per-partition scale AP and emits row-sums
                      via accum_out; eT built by PE-transposing e (2 cyc/row
                      beats redoing fp32 matmuls at 4 cyc/row); content
                      matmul accumulates the two 120-key halves in PSUM;
                      softmax division on DVE; DMA out.

Softmax max-subtraction is skipped: q8n and keys are unit vectors so
beta*sim is in [-2, 2]; exp() there is numerically safe.
"""

import numpy as np

B, S, DQ, H, DK, NS, DV = 4, 65536, 14, 32, 8, 240, 256
N_CORES = 8
N_ROWS = B * S                 # 262144
NC_ROWS = N_ROWS // N_CORES    # 32768
R = 512                        # rows per macro tile
BETA = 2.0

_CACHE = {}


def _build(nc_rows, gelu_name="Gelu"):
    """Build the Bass module (same NEFF on all 8 cores, SPMD over row shards)."""
    from contextlib import ExitStack

    import concourse.bass as bass  # noqa: F401
    import concourse.mybir as mybir
    import concourse.tile as tile
    from concourse import bacc

    f32 = mybir.dt.float32
    AF = mybir.ActivationFunctionType
    mult = mybir.AluOpType.mult
    n_macro = nc_rows // R
    n_chunk = n_macro * 4          # 128-row chunks total (<= 256)
    assert n_chunk <= 256

    nc = bacc.Bacc("TRN2", target_bir_lowering=False, debug=False,
                   num_devices=N_CORES)

    q_d = nc.dram_tensor("q", [nc_rows, DQ], f32, kind="ExternalInput").ap()
    w1s_d = nc.dram_tensor("w1s", [128, 128], f32, kind="ExternalInput").ap()
    w2t_d = nc.dram_tensor("w2t", [H, DK], f32, kind="ExternalInput").ap()
    kt_d = nc.dram_tensor("kt", [DK, NS], f32, kind="ExternalInput").ap()
    va_d = nc.dram_tensor("va", [120, DV], f32, kind="ExternalInput").ap()
    vb_d = nc.dram_tensor("vb", [120, DV], f32, kind="ExternalInput").ap()
    id_d = nc.dram_tensor("ident", [128, 128], f32, kind="ExternalInput").ap()
    b1_d = nc.dram_tensor("b1c", [H, 1], f32, kind="ExternalInput").ap()
    b2_d = nc.dram_tensor("b2c", [DK, 1], f32, kind="ExternalInput").ap()
    eb_d = nc.dram_tensor("ebias", [128, 1], f32, kind="ExternalInput").ap()
    o1_d = nc.dram_tensor("ones81", [DK, 1], f32, kind="ExternalInput").ap()

    attn_d = nc.dram_tensor("attn", [nc_rows, NS], f32, kind="ExternalOutput").ap()
    cont_d = nc.dram_tensor("content", [nc_rows, DV], f32, kind="ExternalOutput").ap()

    with tile.TileContext(nc) as tc, ExitStack() as ctx:
        const = ctx.enter_context(tc.tile_pool(name="const", bufs=1))

        def load_const(name, ap_d, shape):
            t = const.tile(shape, f32, tag=name)
            nc.sync.dma_start(t[:], ap_d)
            return t

        w1s = load_const("w1s", w1s_d, [128, 128])
        w2t = load_const("w2t", w2t_d, [H, DK])
        kt = load_const("kt", kt_d, [DK, NS])
        va = load_const("va", va_d, [120, DV])
        vb = load_const("vb", vb_d, [120, DV])
        ident = load_const("ident", id_d, [128, 128])
        b1t = load_const("b1c", b1_d, [H, 1])
        b2t = load_const("b2c", b2_d, [DK, 1])
        ebt = load_const("ebias", eb_d, [128, 1])
        o81 = load_const("ones81", o1_d, [DK, 1])

        persist = ctx.enter_context(tc.tile_pool(name="persist", bufs=1))
        q8_all = persist.tile([DK, nc_rows], f32, tag="q8all")
        rnc = persist.tile([128, 256], f32, tag="rnc")      # chunk layout

        # ---------------- pass 1: projection ----------------
        with ExitStack() as p1:
            ps_norm = p1.enter_context(tc.tile_pool(name="psn", bufs=1, space="PSUM"))
            norm_ps = ps_norm.tile([128, 256], f32, tag="normp")
            xt_pool = p1.enter_context(tc.tile_pool(name="xt", bufs=3))
            xt2_pool = p1.enter_context(tc.tile_pool(name="xt2", bufs=3))
            h_pool = p1.enter_context(tc.tile_pool(name="hsb", bufs=3))
            sq_pool = p1.enter_context(tc.tile_pool(name="sq", bufs=3))
            ps_x = p1.enter_context(tc.tile_pool(name="psx", bufs=2, space="PSUM"))
            ps_h = p1.enter_context(tc.tile_pool(name="psh", bufs=2, space="PSUM"))
            ps_q = p1.enter_context(tc.tile_pool(name="psq", bufs=2, space="PSUM"))

            for m in range(n_macro):
                xt = xt_pool.tile([128, 128], f32, tag="xt")
                if m < 3:
                    # first touch of each pool slot: clear so the pad cols
                    # (14..31 of each 32-group) are finite; later iterations
                    # inherit finite floats from the previous macro tile.
                    nc.vector.memset(xt[:], 0.0)
                src = q_d[R * m: R * (m + 1), :].rearrange("(g p) k -> p g k", g=4)
                dst = xt[:].rearrange("p (g s) -> p g s", g=4)[:, :, 0:DQ]
                nc.sync.dma_start(dst, src)

                xtp = ps_x.tile([128, 128], f32, tag="xtp")
                nc.tensor.transpose(xtp[:], xt[:], ident[:])
                xt2 = xt2_pool.tile([128, 128], f32, tag="xt2")
                nc.vector.tensor_copy(xt2[:], xtp[:])

                hp = ps_h.tile([H, R], f32, tag="hp")
                for g in range(4):
                    nc.tensor.matmul(hp[:, 128 * g: 128 * (g + 1)],
                                     w1s[:, 32 * g: 32 * (g + 1)], xt2[:],
                                     start=True, stop=True)
                hsb = h_pool.tile([H, R], f32, tag="hsb")
                nc.scalar.activation(hsb[:], hp[:], getattr(AF, gelu_name), bias=b1t[:])

                qp = ps_q.tile([DK, R], f32, tag="qp")
                nc.tensor.matmul(qp[:], w2t[:], hsb[:], start=True, stop=True)
                q8_sl = q8_all[:, R * m: R * (m + 1)]
                nc.scalar.activation(q8_sl, qp[:], AF.Identity, bias=b2t[:])

                sq = sq_pool.tile([DK, R], f32, tag="sq")
                nc.vector.tensor_tensor(sq[:], q8_sl, q8_sl, mult)
                for c in range(4):
                    nc.tensor.matmul(norm_ps[:, 4 * m + c: 4 * m + c + 1],
                                     sq[:, 128 * c: 128 * (c + 1)], o81[:],
                                     start=True, stop=True)

            # ---- prelude: rnorm2 = beta/sqrt(normsq), both layouts ----
            lnv = persist.tile([128, 256], f32, tag="lnv")
            nc.scalar.activation(lnv[:, 0:n_chunk], norm_ps[:, 0:n_chunk], AF.Ln)
            nc.scalar.activation(rnc[:, 0:n_chunk], lnv[:, 0:n_chunk], AF.Exp,
                                 scale=-0.5, bias=ebt[:])

        # ---------------- pass 2: attention ----------------
        with ExitStack() as p2:
            e_pool = p2.enter_context(tc.tile_pool(name="e", bufs=3))
            eT_pool = p2.enter_context(tc.tile_pool(name="eT", bufs=3))
            s_pool = p2.enter_context(tc.tile_pool(name="ssb", bufs=3))
            rec_pool = p2.enter_context(tc.tile_pool(name="rec", bufs=3))
            attn_pool = p2.enter_context(tc.tile_pool(name="attnsb", bufs=3))
            cont_pool = p2.enter_context(tc.tile_pool(name="contsb", bufs=3))
            ps_sr = p2.enter_context(tc.tile_pool(name="pssr", bufs=2, space="PSUM"))
            ps_eT = p2.enter_context(tc.tile_pool(name="pseT", bufs=1, space="PSUM"))
            ps_c = p2.enter_context(tc.tile_pool(name="psc", bufs=1, space="PSUM"))

            for m in range(n_macro):
                q8_sl = q8_all[:, R * m: R * (m + 1)]

                # row sim from raw q8; exp applies rnorm2 scale, emits row sums
                e_sb = e_pool.tile([128, 4 * NS], f32, tag="e")
                s_sb = s_pool.tile([128, 4], f32, tag="ssb")
                for half in range(2):
                    sr = ps_sr.tile([128, 2 * NS], f32, tag="sr")
                    for i in range(2):
                        c = 2 * half + i
                        nc.tensor.matmul(sr[:, NS * i: NS * (i + 1)],
                                         q8_sl[:, 128 * c: 128 * (c + 1)], kt[:],
                                         start=True, stop=True)
                    for i in range(2):
                        c = 2 * half + i
                        col = 4 * m + c
                        nc.scalar.activation(
                            e_sb[:, NS * c: NS * (c + 1)],
                            sr[:, NS * i: NS * (i + 1)], AF.Exp,
                            scale=rnc[:, col: col + 1],
                            accum_out=s_sb[:, c: c + 1])

                # eT chunks via PE transpose of e (2 cyc/row beats fp32 mm)
                eTs = {}
                for c in range(4):
                    for h in range(2):
                        etp = ps_eT.tile([120, 128], f32, tag=f"etp{h}")
                        nc.tensor.transpose(
                            etp[:],
                            e_sb[:, NS * c + 120 * h: NS * c + 120 * (h + 1)],
                            ident[:])
                        et = eT_pool.tile([120, 128], f32, tag=f"eT{h}{c % 2}")
                        if (c + h) % 2 == 0:
                            nc.scalar.copy(et[:], etp[:])
                        else:
                            nc.vector.tensor_copy(et[:], etp[:])
                        eTs[(h, c)] = et

                # content = eT.T @ values (240 keys split 120+120)
                cps = ps_c.tile([128, 2048], f32, tag="cps")
                for c in range(4):
                    out_sl = cps[:, 512 * c: 512 * c + DV]
                    nc.tensor.matmul(out_sl, eTs[(0, c)][:], va[:],
                                     start=True, stop=False, skip_group_check=True)
                    nc.tensor.matmul(out_sl, eTs[(1, c)][:], vb[:],
                                     start=False, stop=True, skip_group_check=True)

                rec = rec_pool.tile([128, 4], f32, tag="rec")
                nc.vector.reciprocal(rec[:], s_sb[:])

                attn_sb = attn_pool.tile([128, 4 * NS], f32, tag="attnsb")
                for c in range(4):
                    nc.vector.tensor_scalar_mul(
                        attn_sb[:, NS * c: NS * (c + 1)],
                        e_sb[:, NS * c: NS * (c + 1)], rec[:, c: c + 1])

                cont_sb = cont_pool.tile([128, 4 * DV], f32, tag="contsb")
                cps_v = cps[:].rearrange("p (c v) -> p c v", c=4)
                rec_b = rec[:].unsqueeze(2).broadcast_to([128, 4, DV])
                nc.vector.tensor_tensor(
                    cont_sb[:].rearrange("p (c v) -> p c v", c=4),
                    cps_v[:, :, 0:DV], rec_b, mult)

                a_dst = attn_d[R * m: R * (m + 1), :].rearrange(
                    "(c p) k -> p c k", c=4)
                nc.sync.dma_start(a_dst, attn_sb[:].rearrange(
                    "p (c k) -> p c k", c=4))
                c_dst = cont_d[R * m: R * (m + 1), :].rearrange(
                    "(c p) v -> p c v", c=4)
                nc.sync.dma_start(c_dst, cont_sb[:].rearrange(
                    "p (c v) -> p c v", c=4))

    nc.finalize()
    return nc


def _host_consts(W1, b1, W2, b2, values, keys_n):
    f = np.float32
    w1s = np.zeros((128, 128), f)
    for g in range(4):
        w1s[32 * g: 32 * g + DQ, 32 * g: 32 * (g + 1)] = W1.T.astype(f)
    w2t = np.ascontiguousarray(W2.T, dtype=f)
    kt = np.ascontiguousarray(keys_n.T, dtype=f)
    va = np.ascontiguousarray(values[0:120], dtype=f)
    vb = np.ascontiguousarray(values[120:240], dtype=f)
    ident = np.eye(128, dtype=f)
    b1c = np.ascontiguousarray(b1.reshape(H, 1), dtype=f)
    b2c = np.ascontiguousarray(b2.reshape(DK, 1), dtype=f)
    ebias = np.full((128, 1), np.log(BETA), f)
    ones81 = np.ones((DK, 1), f)
    return {"w1s": w1s, "w2t": w2t, "kt": kt, "va": va, "vb": vb,
            "ident": ident, "b1c": b1c, "b2c": b2c, "ebias": ebias,
            "ones81": ones81}


def kernel(query, W1, b1, W2, b2, values, keys_n):
    from concourse.bass_utils import run_bass_kernel_spmd

    x = np.ascontiguousarray(np.asarray(query, dtype=np.float32).reshape(-1, DQ))
    consts = _host_consts(np.asarray(W1), np.asarray(b1), np.asarray(W2),
                          np.asarray(b2), np.asarray(values), np.asarray(keys_n))

    if "nc" not in _CACHE:
        _CACHE["nc"] = _build(NC_ROWS)
    nc = _CACHE["nc"]

    in_maps = []
    for c in range(N_CORES):
        m = dict(consts)
        m["q"] = np.ascontiguousarray(x[c * NC_ROWS:(c + 1) * NC_ROWS])
        in_maps.append(m)

    res = run_bass_kernel_spmd(nc, in_maps, core_ids=list(range(N_CORES)))
    _CACHE["last_res"] = res
    content = np.concatenate([r["content"] for r in res.results], axis=0)
    attn = np.concatenate([r["attn"] for r in res.results], axis=0)
    return content, attn


# revision 28
# speedup vs baseline: 1.0471x; 1.0471x over previous
"""Trainium2 Bass kernel for nn_EpisodicMemory (scatter_memory).

Computation (reference):
  x  = query.reshape(-1, 14)                      # [N, 14], N = 4*65536
  h  = gelu(x @ W1.T + b1)                        # [N, 32]
  q8 = h @ W2.T + b2                              # [N, 8]
  q8n= q8 / max(||q8||, 1e-12)
  sim= q8n @ keys_n.T                             # [N, 240]
  attn = softmax(2.0 * sim)                       # [N, 240]
  content = attn @ values                         # [N, 256]
  return content, attn

Sharding: data-parallel over rows, N/8 = 32768 rows per NeuronCore.

Per-core dataflow (two ACT-table passes to avoid gelu<->exp table thrash):
  pass 1 (gelu set):  DMA x -> PE transpose -> mm1 -> gelu -> mm2
                      -> q8 parked in SBUF [8, 32768]; per-chunk |q8|^2
                      collected into one PSUM [128, 256] tile (col = chunk).
  prelude:            rnorm2 = beta/sqrt(normsq) = exp(-0.5*ln(normsq)+ln beta)
                      batched for all rows: one Ln + one Exp total.
  pass 2 (ln/exp set): sim rows [128,240] x4 from raw q8; exp applies
                      the rnorm2 per-partition scale AP and emits row-sums
                      via accum_out; eT built by PE-transposing e (2 cyc/row
                      beats redoing fp32 matmuls at 4 cyc/row); content
                      matmul accumulates the two 120-key halves in PSUM;
                      softmax division on DVE; DMA out.

Softmax max-subtraction is skipped: q8n and keys are unit vectors so
beta*sim is in [-2, 2]; exp() there is numerically safe.
"""

import numpy as np

B, S, DQ, H, DK, NS, DV = 4, 65536, 14, 32, 8, 240, 256
N_CORES = 8
N_ROWS = B * S                 # 262144
NC_ROWS = N_ROWS // N_CORES    # 32768
R = 512                        # rows per macro tile
BETA = 2.0

_CACHE = {}


def _build(nc_rows, gelu_name="Gelu"):
    """Build the Bass module (same NEFF on all 8 cores, SPMD over row shards)."""
    from contextlib import ExitStack

    import concourse.bass as bass  # noqa: F401
    import concourse.mybir as mybir
    import concourse.tile as tile
    from concourse import bacc

    f32 = mybir.dt.float32
    AF = mybir.ActivationFunctionType
    mult = mybir.AluOpType.mult
    n_macro = nc_rows // R
    n_chunk = n_macro * 4          # 128-row chunks total (<= 256)
    assert n_chunk <= 256

    nc = bacc.Bacc("TRN2", target_bir_lowering=False, debug=False,
                   num_devices=N_CORES)

    q_d = nc.dram_tensor("q", [nc_rows, DQ], f32, kind="ExternalInput").ap()
    w1s_d = nc.dram_tensor("w1s", [128, 128], f32, kind="ExternalInput").ap()
    w2t_d = nc.dram_tensor("w2t", [H, DK], f32, kind="ExternalInput").ap()
    kt_d = nc.dram_tensor("kt", [DK, NS], f32, kind="ExternalInput").ap()
    va_d = nc.dram_tensor("va", [120, DV], f32, kind="ExternalInput").ap()
    vb_d = nc.dram_tensor("vb", [120, DV], f32, kind="ExternalInput").ap()
    id_d = nc.dram_tensor("ident", [128, 128], f32, kind="ExternalInput").ap()
    b1_d = nc.dram_tensor("b1c", [H, 1], f32, kind="ExternalInput").ap()
    b2_d = nc.dram_tensor("b2c", [DK, 1], f32, kind="ExternalInput").ap()
    eb_d = nc.dram_tensor("ebias", [128, 1], f32, kind="ExternalInput").ap()
    o1_d = nc.dram_tensor("ones81", [DK, 1], f32, kind="ExternalInput").ap()

    attn_d = nc.dram_tensor("attn", [nc_rows, NS], f32, kind="ExternalOutput").ap()
    cont_d = nc.dram_tensor("content", [nc_rows, DV], f32, kind="ExternalOutput").ap()

    with tile.TileContext(nc) as tc, ExitStack() as ctx:
        const = ctx.enter_context(tc.tile_pool(name="const", bufs=1))

        def load_const(name, ap_d, shape):
            t = const.tile(shape, f32, tag=name)
            nc.sync.dma_start(t[:], ap_d)
            return t

        w1s = load_const("w1s", w1s_d, [128, 128])
        w2t = load_const("w2t", w2t_d, [H, DK])
        kt = load_const("kt", kt_d, [DK, NS])
        va = load_const("va", va_d, [120, DV])
        vb = load_const("vb", vb_d, [120, DV])
        ident = load_const("ident", id_d, [128, 128])
        b1t = load_const("b1c", b1_d, [H, 1])
        b2t = load_const("b2c", b2_d, [DK, 1])
        ebt = load_const("ebias", eb_d, [128, 1])
        o81 = load_const("ones81", o1_d, [DK, 1])

        persist = ctx.enter_context(tc.tile_pool(name="persist", bufs=1))
        q8_all = persist.tile([DK, nc_rows], f32, tag="q8all")
        rnc = persist.tile([128, 256], f32, tag="rnc")      # chunk layout

        # ---------------- pass 1: projection ----------------
        with ExitStack() as p1:
            ps_norm = p1.enter_context(tc.tile_pool(name="psn", bufs=1, space="PSUM"))
            norm_ps = ps_norm.tile([128, 256], f32, tag="normp")
            xt_pool = p1.enter_context(tc.tile_pool(name="xt", bufs=5))
            xt2_pool = p1.enter_context(tc.tile_pool(name="xt2", bufs=5))
            h_pool = p1.enter_context(tc.tile_pool(name="hsb", bufs=3))
            sq_pool = p1.enter_context(tc.tile_pool(name="sq", bufs=3))
            ps_x = p1.enter_context(tc.tile_pool(name="psx", bufs=4, space="PSUM"))
            ps_h = p1.enter_context(tc.tile_pool(name="psh", bufs=2, space="PSUM"))
            ps_q = p1.enter_context(tc.tile_pool(name="psq", bufs=1, space="PSUM"))

            for m in range(n_macro):
                xt = xt_pool.tile([128, 128], f32, tag="xt")
                if m < 5:
                    # first touch of each pool slot: clear so the pad cols
                    # (14..31 of each 32-group) are finite; later iterations
                    # inherit finite floats from the previous macro tile.
                    nc.vector.memset(xt[:], 0.0)
                src = q_d[R * m: R * (m + 1), :].rearrange("(g p) k -> p g k", g=4)
                dst = xt[:].rearrange("p (g s) -> p g s", g=4)[:, :, 0:DQ]
                nc.sync.dma_start(dst, src)

                xtp = ps_x.tile([128, 128], f32, tag="xtp")
                nc.tensor.transpose(xtp[:], xt[:], ident[:])
                xt2 = xt2_pool.tile([128, 128], f32, tag="xt2")
                nc.vector.tensor_copy(xt2[:], xtp[:])

                hp = ps_h.tile([H, R], f32, tag="hp")
                for g in range(4):
                    nc.tensor.matmul(hp[:, 128 * g: 128 * (g + 1)],
                                     w1s[:, 32 * g: 32 * (g + 1)], xt2[:],
                                     start=True, stop=True)
                hsb = h_pool.tile([H, R], f32, tag="hsb")
                nc.scalar.activation(hsb[:], hp[:], getattr(AF, gelu_name), bias=b1t[:])

                qp = ps_q.tile([DK, R], f32, tag="qp")
                nc.tensor.matmul(qp[:], w2t[:], hsb[:], start=True, stop=True)
                q8_sl = q8_all[:, R * m: R * (m + 1)]
                nc.scalar.activation(q8_sl, qp[:], AF.Identity, bias=b2t[:])

                sq = sq_pool.tile([DK, R], f32, tag="sq")
                nc.vector.tensor_tensor(sq[:], q8_sl, q8_sl, mult)
                for c in range(4):
                    nc.tensor.matmul(norm_ps[:, 4 * m + c: 4 * m + c + 1],
                                     sq[:, 128 * c: 128 * (c + 1)], o81[:],
                                     start=True, stop=True)

            # ---- prelude: rnorm2 = beta/sqrt(normsq), both layouts ----
            lnv = persist.tile([128, 256], f32, tag="lnv")
            nc.scalar.activation(lnv[:, 0:n_chunk], norm_ps[:, 0:n_chunk], AF.Ln)
            nc.scalar.activation(rnc[:, 0:n_chunk], lnv[:, 0:n_chunk], AF.Exp,
                                 scale=-0.5, bias=ebt[:])

        # ---------------- pass 2: attention ----------------
        with ExitStack() as p2:
            e_pool = p2.enter_context(tc.tile_pool(name="e", bufs=4))
            eT_pool = p2.enter_context(tc.tile_pool(name="eT", bufs=3))
            s_pool = p2.enter_context(tc.tile_pool(name="ssb", bufs=3))
            rec_pool = p2.enter_context(tc.tile_pool(name="rec", bufs=3))
            attn_pool = p2.enter_context(tc.tile_pool(name="attnsb", bufs=4))
            cont_pool = p2.enter_context(tc.tile_pool(name="contsb", bufs=4))
            ps_sr = p2.enter_context(tc.tile_pool(name="pssr", bufs=2, space="PSUM"))
            ps_eT = p2.enter_context(tc.tile_pool(name="pseT", bufs=1, space="PSUM"))
            ps_c = p2.enter_context(tc.tile_pool(name="psc", bufs=1, space="PSUM"))

            for m in range(n_macro):
                q8_sl = q8_all[:, R * m: R * (m + 1)]

                # row sim from raw q8; exp applies rnorm2 scale, emits row sums
                e_sb = e_pool.tile([128, 4 * NS], f32, tag="e")
                s_sb = s_pool.tile([128, 4], f32, tag="ssb")
                for half in range(2):
                    sr = ps_sr.tile([128, 2 * NS], f32, tag="sr")
                    for i in range(2):
                        c = 2 * half + i
                        nc.tensor.matmul(sr[:, NS * i: NS * (i + 1)],
                                         q8_sl[:, 128 * c: 128 * (c + 1)], kt[:],
                                         start=True, stop=True)
                    for i in range(2):
                        c = 2 * half + i
                        col = 4 * m + c
                        nc.scalar.activation(
                            e_sb[:, NS * c: NS * (c + 1)],
                            sr[:, NS * i: NS * (i + 1)], AF.Exp,
                            scale=rnc[:, col: col + 1],
                            accum_out=s_sb[:, c: c + 1])

                # eT chunks via PE transpose of e (2 cyc/row beats fp32 mm)
                eTs = {}
                for c in range(4):
                    for h in range(2):
                        etp = ps_eT.tile([120, 128], f32, tag=f"etp{h}")
                        nc.tensor.transpose(
                            etp[:],
                            e_sb[:, NS * c + 120 * h: NS * c + 120 * (h + 1)],
                            ident[:])
                        et = eT_pool.tile([120, 128], f32, tag=f"eT{h}{c % 2}")
                        if (c + h) % 2 == 0:
                            nc.scalar.copy(et[:], etp[:])
                        else:
                            nc.vector.tensor_copy(et[:], etp[:])
                        eTs[(h, c)] = et

                # content = eT.T @ values (240 keys split 120+120)
                cps = ps_c.tile([128, 2048], f32, tag="cps")
                for c in range(4):
                    out_sl = cps[:, 512 * c: 512 * c + DV]
                    nc.tensor.matmul(out_sl, eTs[(0, c)][:], va[:],
                                     start=True, stop=False, skip_group_check=True)
                    nc.tensor.matmul(out_sl, eTs[(1, c)][:], vb[:],
                                     start=False, stop=True, skip_group_check=True)

                rec = rec_pool.tile([128, 4], f32, tag="rec")
                nc.vector.reciprocal(rec[:], s_sb[:])

                attn_sb = attn_pool.tile([128, 4 * NS], f32, tag="attnsb")
                for c in range(4):
                    nc.vector.tensor_scalar_mul(
                        attn_sb[:, NS * c: NS * (c + 1)],
                        e_sb[:, NS * c: NS * (c + 1)], rec[:, c: c + 1])

                cont_sb = cont_pool.tile([128, 4 * DV], f32, tag="contsb")
                cps_v = cps[:].rearrange("p (c v) -> p c v", c=4)
                rec_b = rec[:].unsqueeze(2).broadcast_to([128, 4, DV])
                nc.vector.tensor_tensor(
                    cont_sb[:].rearrange("p (c v) -> p c v", c=4),
                    cps_v[:, :, 0:DV], rec_b, mult)

                a_dst = attn_d[R * m: R * (m + 1), :].rearrange(
                    "(c p) k -> p c k", c=4)
                nc.sync.dma_start(a_dst, attn_sb[:].rearrange(
                    "p (c k) -> p c k", c=4))
                c_dst = cont_d[R * m: R * (m + 1), :].rearrange(
                    "(c p) v -> p c v", c=4)
                nc.sync.dma_start(c_dst, cont_sb[:].rearrange(
                    "p (c v) -> p c v", c=4))

    nc.finalize()
    return nc


def _host_consts(W1, b1, W2, b2, values, keys_n):
    f = np.float32
    w1s = np.zeros((128, 128), f)
    for g in range(4):
        w1s[32 * g: 32 * g + DQ, 32 * g: 32 * (g + 1)] = W1.T.astype(f)
    w2t = np.ascontiguousarray(W2.T, dtype=f)
    kt = np.ascontiguousarray(keys_n.T, dtype=f)
    va = np.ascontiguousarray(values[0:120], dtype=f)
    vb = np.ascontiguousarray(values[120:240], dtype=f)
    ident = np.eye(128, dtype=f)
    b1c = np.ascontiguousarray(b1.reshape(H, 1), dtype=f)
    b2c = np.ascontiguousarray(b2.reshape(DK, 1), dtype=f)
    ebias = np.full((128, 1), np.log(BETA), f)
    ones81 = np.ones((DK, 1), f)
    return {"w1s": w1s, "w2t": w2t, "kt": kt, "va": va, "vb": vb,
            "ident": ident, "b1c": b1c, "b2c": b2c, "ebias": ebias,
            "ones81": ones81}


def kernel(query, W1, b1, W2, b2, values, keys_n):
    from concourse.bass_utils import run_bass_kernel_spmd

    x = np.ascontiguousarray(np.asarray(query, dtype=np.float32).reshape(-1, DQ))
    consts = _host_consts(np.asarray(W1), np.asarray(b1), np.asarray(W2),
                          np.asarray(b2), np.asarray(values), np.asarray(keys_n))

    if "nc" not in _CACHE:
        _CACHE["nc"] = _build(NC_ROWS)
    nc = _CACHE["nc"]

    in_maps = []
    for c in range(N_CORES):
        m = dict(consts)
        m["q"] = np.ascontiguousarray(x[c * NC_ROWS:(c + 1) * NC_ROWS])
        in_maps.append(m)

    res = run_bass_kernel_spmd(nc, in_maps, core_ids=list(range(N_CORES)))
    _CACHE["last_res"] = res
    content = np.concatenate([r["content"] for r in res.results], axis=0)
    attn = np.concatenate([r["attn"] for r in res.results], axis=0)
    return content, attn
